# revision 28
# baseline (speedup 1.0000x reference)
"""Trainium2 Bass kernel for nn_AgeUGP_v2 (gnn_message_passing).

Reference pipeline:
  snp_h[b,n,f] = snp[b,n] * filters[f,n]
  gathered     = snp_h[:, snp_ids, :]
  per_gene     = segment_sum(gathered, node_seg)   # node_seg sorted
  sample_h     = per_gene.mean(-1)
  h1 = sample_h @ W1 ... tiny MLP tail

Algebraic collapse: the filter axis F is only averaged at the end, so
  sample_h[b,g] = sum_{i in seg g} snp[b, id_i] * fbar[id_i],
  fbar = mean(filters, axis=0).

Device strategy v2 (8 NeuronCores, genes sharded across cores):
  - Per-core SNP COMPACTION: each core's nodes reference ~197k unique SNPs
    (of 500k); the host selects and orders just those (pure permutation),
    split into 64 chunks of Kc.  4 table passes; pass T holds 16 chunks on
    128 partitions: partition p = 16g + 8h + b carries chunk 16T+g+8h,
    batch b.
  - ZERO-JUNK split tables: each partition's gather table is [2*Kc+2] with
    its chunk's values v = snp * fbar at [h*Kc : (h+1)*Kc] and ZEROS
    elsewhere (zeroed once per buffer; DMAs only rewrite data halves).
    An index in [0,Kc) reads chunk A's value on h=0 lanes and exact 0 on
    h=1 lanes (and vice versa), so the 16-lane shared-index junk vanishes
    arithmetically: A/B contributions merge into ONE gene segment.
  - fbar is produced fused on device: a bf16 host-permuted copy of filters
    is hit with 1/8-valued mean+replicate PE matmuls (routeA/routeB) whose
    PSUM output multiplies the table halves on DVE (zeros stay zero).
  - One gpsimd ap_gather per pass streams both chunks' nodes gene-ordered
    (per-gene counts padded to EVEN with pads pointing at the zero column).
    A DVE tensor_tensor_scan with data0/data1 = even/odd stride-2 views
    forms PAIR prefix sums in place (halving scan and extraction size); a
    second ap_gather extracts one prefix per gene END; one adjacent
    difference gives per-(gene,half,batch) sums; a single sel matmul per
    gene tile folds halves+lanes into sample_h [gene, batch] in PSUM.
  - PE matmul with the core's W1 shard (bf16) -> partial h1 [8, 1024];
    host sums the 8 partials and runs the tiny MLP tail (0.01% of FLOPs).
Emission is software-pipelined (gather p+1 ahead of pass-p tail; tables
manually double-buffered so the zero halves persist across passes).
"""

import numpy as np

B = 8
N_SNPS = 500000
N_NODES = 2000000
N_GENES = 20000
N_FILT = 8
N_CORES = 8
BN_EPS = 1e-5

_P = 128
_NCHUNK = 64  # compact SNP chunks per core
_NTAB = 4  # table passes
_EPAD = 16


def make_cfg(Kc, J, n_genes=N_GENES, n_cores=N_CORES, d1=1024):
    gpc = n_genes // n_cores
    jt = -(-gpc // _P)
    gpad = jt * _P
    ns = gpc + 1  # boundaries: dummy zero + one end per gene
    nspad = -(-ns // _EPAD) * _EPAD
    assert J % 16 == 0 and J % 4 == 0
    assert 2 * Kc + 2 <= 2**15, "gather table exceeds num_elems limit"
    assert J <= 32752, "stream length exceeds int16 index range"
    return dict(
        Kc=Kc, J=J, gpc=gpc, gpad=gpad, jt=jt, d1=d1, ns=ns, nspad=nspad,
        n_cores=n_cores,
    )


# ---------------------------------------------------------------- device program
def build_program(cfg):
    import concourse.bass as bass
    import concourse.bacc as bacc
    import concourse.mybir as mybir
    import concourse.tile as tile

    fp32 = mybir.dt.float32
    bf16 = mybir.dt.bfloat16
    i16 = mybir.dt.int16

    Kc, J = cfg["Kc"], cfg["J"]
    jt, d1 = cfg["jt"], cfg["d1"]
    gpc, gpad, nspad = cfg["gpc"], cfg["gpad"], cfg["nspad"]
    TW = 2 * Kc + 2  # table width: [A-half | B-half | zero col pair]
    JH = J // 2

    nc = bacc.Bacc(
        "TRN2", target_bir_lowering=False, debug=False, num_devices=cfg["n_cores"]
    )

    snp_in = nc.dram_tensor("snp_perm", [_P, _NTAB * TW], fp32, kind="ExternalInput")
    filt_in = nc.dram_tensor("filt_perm", [_P, _NTAB * Kc], bf16, kind="ExternalInput")
    gidx_in = nc.dram_tensor("gidx", [_P, _NTAB * (J // 16)], i16, kind="ExternalInput")
    eidx_in = nc.dram_tensor(
        "eidx", [_P, _NTAB * (nspad // 16)], i16, kind="ExternalInput"
    )
    sel_in = nc.dram_tensor("sel", [_P, 8], bf16, kind="ExternalInput")
    route_in = nc.dram_tensor("mroute", [_P, 2 * _P], bf16, kind="ExternalInput")
    w1_in = nc.dram_tensor("w1c", [_P, jt * d1], bf16, kind="ExternalInput")
    h1_out = nc.dram_tensor("h1p", [B, d1], fp32, kind="ExternalOutput")

    rc = Kc // 4  # route/mul block width (2-bank PSUM tiles)
    assert rc * 4 == Kc and rc * 4 <= 4096

    with tile.TileContext(nc) as tc:
        with (
            tc.tile_pool(name="per", bufs=1) as perpool,
            tc.tile_pool(name="tab", bufs=2) as tabpool,
            tc.tile_pool(name="gs", bufs=2) as gspool,
            tc.tile_pool(name="ft", bufs=1) as ftpool,
            tc.tile_pool(name="ex", bufs=2) as expool,
            tc.tile_pool(name="w1", bufs=5) as w1pool,
            tc.tile_pool(name="ps", bufs=2, space="PSUM") as pspool,
            tc.tile_pool(name="psw", bufs=1, space="PSUM") as pswpool,
            tc.tile_pool(name="psh", bufs=2, space="PSUM") as pshpool,
        ):
            DMAQ, DVEQ, POOLQ, PEQ = [], [], [], []

            route = perpool.tile([_P, 2 * _P], bf16, tag="route")
            route_d = nc.sync.dma_start(route[:], route_in.ap())
            sel8 = perpool.tile([_P, 8], bf16, tag="sel8")
            sel_d = nc.sync.dma_start(sel8[:], sel_in.ap())

            # sample_h accumulator [gene-tile, (t, b)]
            sh = perpool.tile([_P, jt * B], fp32, tag="sh")
            DVEQ.append(nc.vector.memset(sh[:], 0.0))
            # dd holds per-(lane,gene) sums; pad cols stay zero forever
            dd = perpool.tile([_P, gpad], bf16, tag="dd")
            DVEQ.append(nc.vector.memset(dd[:], 0.0))

            from concourse.instruction_name_ordered_set import (
                InstructionNameOrderedSet,
            )

            def pin(later, earlier):
                """Same-engine order pin (no runtime semaphore)."""
                s = InstructionNameOrderedSet()
                s.add(earlier.ins.name)
                later.ins.add_nosync_dependencies_from(s)

            vtabs = {}

            def emit_table(T):
                # DRAM rows carry the zero-split layout already (data half +
                # zero half per partition parity): two wide DMAs per pass,
                # the zero columns arrive as part of the transfer
                vtab = tabpool.tile([_P, TW], fp32, tag="vtab", name=f"vtab{T}")
                ft = ftpool.tile([_P, Kc], bf16, tag="ftl", name=f"ftl{T}")
                dmas = [
                    nc.sync.dma_start(
                        ft[:], filt_in.ap()[:, T * Kc : (T + 1) * Kc]
                    ),
                    nc.sync.dma_start(
                        vtab[:, 0:Kc], snp_in.ap()[:, T * TW : T * TW + Kc]
                    ),
                    nc.sync.dma_start(
                        vtab[:, Kc:TW], snp_in.ap()[:, T * TW + Kc : (T + 1) * TW]
                    ),
                ]
                muls, prs = [], []
                for half in range(2):
                    for blk in range(4):
                        pr = pspool.tile([_P, rc], fp32, tag="pr", name="pr")
                        prs.append(
                            nc.tensor.matmul(
                                pr[:],
                                route[:, half * _P : (half + 1) * _P],
                                ft[:, blk * rc : (blk + 1) * rc],
                                start=True, stop=True,
                            )
                        )
                        ks = slice(half * Kc + blk * rc, half * Kc + (blk + 1) * rc)
                        muls.append(
                            nc.vector.tensor_mul(vtab[:, ks], vtab[:, ks], pr[:])
                        )
                vtabs[T] = vtab
                return dict(dmas=dmas, muls=muls, prs=prs)

            # index streams prefetched once (each pass's stream is its own
            # tile: ap_gather idx APs must start at a tile base)
            gidx_t, eidx_t = {}, {}

            def prefetch_idx():
                dmas = []
                for p in range(_NTAB):
                    g = perpool.tile([_P, J // 16], i16, tag=f"gidx{p}")
                    dmas.append(
                        nc.sync.dma_start(
                            g[:],
                            gidx_in.ap()[:, p * (J // 16) : (p + 1) * (J // 16)],
                        )
                    )
                    gidx_t[p] = g
                    e = perpool.tile([_P, nspad // 16], i16, tag=f"eidx{p}")
                    dmas.append(
                        nc.sync.dma_start(
                            e[:],
                            eidx_in.ap()[
                                :, p * (nspad // 16) : (p + 1) * (nspad // 16)
                            ],
                        )
                    )
                    eidx_t[p] = e
                return dmas

            def emit_gather(p):
                gidx = gidx_t[p]
                gout = gspool.tile([_P, J], fp32, tag="gout", name=f"gout{p}")
                g1 = nc.gpsimd.ap_gather(
                    gout[:], vtabs.pop(p)[:], gidx[:],
                    channels=_P, num_elems=TW, d=1, num_idxs=J,
                )
                return gout, g1

            def emit_scan_extract(p, gout):
                # pair prefix scan, in place into the first half (writes
                # trail the stride-2 reads)
                gall = gout[:]
                even = bass.AP(gall.tensor, gall.offset, [gall.ap[0], [2, JH]])
                godd = gout[:, 1:]
                odd = bass.AP(godd.tensor, godd.offset, [godd.ap[0], [2, JH]])
                sc = nc.vector.tensor_tensor_scan(
                    gout[:, :JH], even, odd, 0.0,
                    op0=mybir.AluOpType.add, op1=mybir.AluOpType.add,
                )
                eidx = eidx_t[p]
                ex = expool.tile([_P, nspad], fp32, tag="ex", name=f"ex{p}")
                g2 = nc.gpsimd.ap_gather(
                    ex[:], gout[:, :JH], eidx[:],
                    channels=_P, num_elems=JH, d=1, num_idxs=nspad,
                )
                return sc, g2, ex

            def emit_reduce(p, ex):
                sub = nc.vector.tensor_sub(
                    dd[:, :gpc], ex[:, 1 : gpc + 1], ex[:, :gpc]
                )
                pst = pshpool.tile([_P, jt * B], fp32, tag="pst", name="pst")
                mms = []
                for t in range(jt):
                    mms.append(
                        nc.tensor.matmul(
                            pst[:, t * B : (t + 1) * B],
                            dd[:, t * _P : (t + 1) * _P],
                            sel8[:],
                            start=True, stop=True,
                        )
                    )
                add = nc.vector.tensor_add(sh[:], sh[:], pst[:])
                return sub, add, mms

            # Emission with a fully pinned per-engine static schedule (the
            # tile scheduler's internal timing model mispredicts DMA
            # completion and otherwise serializes the pipeline):
            #   Pool: g1(0) g1(1) g2(0) g1(2) g2(1) g1(3) g2(2) g2(3)
            #   DVE : muls(0) muls(1) scan0 muls2 scan1 muls3 dd0 scan2
            #         dd1 scan3 dd2 dd3 shb copies
            #   DMA : tables in pass order, idx after table0, W1 after B3
            wgrp = 4 if jt % 4 == 0 else 1  # K-tiles per W1 load
            tabs = {0: emit_table(0)}
            idx_dmas = prefetch_idx()
            tabs[1] = emit_table(1)
            w1ts, w1dmas = [], []
            for jg in range(jt // wgrp):
                w1t = w1pool.tile([_P, wgrp * d1], bf16, tag="w1t", name=f"w1t{jg}")
                w1dmas.append(
                    nc.sync.dma_start(
                        w1t[:], w1_in.ap()[:, jg * wgrp * d1 : (jg + 1) * wgrp * d1]
                    )
                )
                w1ts.append(w1t)
            gouts, g1i, scans, g2i, exs = {}, {}, {}, {}, {}
            reds = {}
            gouts[0], g1i[0] = emit_gather(0)
            for p in range(_NTAB):
                if p + 1 < _NTAB:
                    gouts[p + 1], g1i[p + 1] = emit_gather(p + 1)
                scans[p], g2i[p], exs[p] = emit_scan_extract(p, gouts.pop(p))
                if p + 2 < _NTAB:
                    tabs[p + 2] = emit_table(p + 2)
                if p >= 1:
                    reds[p - 1] = emit_reduce(p - 1, exs.pop(p - 1))
            reds[_NTAB - 1] = emit_reduce(_NTAB - 1, exs.pop(_NTAB - 1))

            shb = perpool.tile([_P, jt * B], bf16, tag="shb")
            shb_c = nc.vector.tensor_copy(shb[:], sh[:])

            # ---- W1 matmul: accumulate over jt K-tiles --------------------
            n_half = min(512, d1)
            n_banks = -(-d1 // n_half)
            pss = []
            for nb in range(n_banks):
                pst = pswpool.tile([_P, n_half], fp32, tag=f"ps{nb}", name=f"ps{nb}")
                pss.append(pst)
            w1mms = []
            for jg in range(jt // wgrp):
                w1t = w1ts[jg]
                for jl in range(wgrp):
                    j = jg * wgrp + jl
                    lhsT = shb[:, j * B : (j + 1) * B]
                    for nb in range(n_banks):
                        w1mms.append(
                            nc.tensor.matmul(
                                pss[nb][:B, :],
                                lhsT,
                                w1t[:, jl * d1 + nb * n_half : jl * d1 + (nb + 1) * n_half],
                                start=(j == 0),
                                stop=(j == jt - 1),
                            )
                        )

            h1 = perpool.tile([B, d1], fp32, tag="h1")
            h1copies = []
            for nb in range(n_banks):
                h1copies.append(
                    nc.vector.tensor_copy(
                        h1[:, nb * n_half : (nb + 1) * n_half], pss[nb][:B, :]
                    )
                )
            DMAQ_out = nc.sync.dma_start(h1_out.ap(), h1[:])

            # ---------------- static order pins ----------------
            def chain(seq):
                for a, b in zip(seq, seq[1:]):
                    pin(b, a)

            # DMA: table 0 first (route/sel slot in after its A half so the
            # first multiplies start as early as possible), then table 1,
            # index streams, remaining tables, W1, output
            ft0, A0, B0 = tabs[0]["dmas"]
            DMAQ += [ft0, A0, route_d, sel_d, B0]
            DMAQ += tabs[1]["dmas"] + idx_dmas
            DMAQ += tabs[2]["dmas"] + tabs[3]["dmas"] + w1dmas + [DMAQ_out]
            chain(DMAQ)
            # Pool: strict alternation, next gather ahead of extraction
            POOLQ += [g1i[0], g1i[1], g2i[0], g1i[2], g2i[1], g1i[3],
                      g2i[2], g2i[3]]
            chain(POOLQ)
            # DVE: table muls for p+2 between scan(p) and scan(p+1);
            # reduces as soon as their extraction lands
            DVEQ += tabs[0]["muls"] + tabs[1]["muls"]
            DVEQ += [scans[0]] + tabs[2]["muls"] + [scans[1]] + tabs[3]["muls"]
            DVEQ += [reds[0][0], reds[0][1], scans[2], reds[1][0], reds[1][1]]
            DVEQ += [scans[3], reds[2][0], reds[2][1], reds[3][0], reds[3][1]]
            DVEQ += [shb_c] + h1copies
            chain(DVEQ)
            # PE: route matmuls in pass order, then sel matmuls, then W1
            PEQ += tabs[0]["prs"] + tabs[1]["prs"] + tabs[2]["prs"]
            PEQ += tabs[3]["prs"]
            for p in range(_NTAB):
                PEQ += reds[p][2]
            PEQ += w1mms
            chain(PEQ)

    nc.compile()
    return nc


# ---------------------------------------------------------------- host side
def _wrap16(streams):
    """[8, J] per-group streams -> [128, J//16] wrapped-16 layout."""
    ngrp, J = streams.shape
    assert ngrp == 8 and J % 16 == 0
    out = np.zeros((_P, J // 16), streams.dtype)
    for g in range(8):
        out[g * 16 : (g + 1) * 16, :] = streams[g].reshape(J // 16, 16).T
    return out


def _core_slices(snp_ids, node_seg):
    ids = np.asarray(snp_ids).astype(np.int64)
    seg = np.asarray(node_seg).astype(np.int64)
    gpc = N_GENES // N_CORES
    gene_starts = np.searchsorted(seg, np.arange(0, N_GENES + 1, gpc))
    return ids, seg, gpc, gene_starts


def _bucket_counts(ids_c, gene_c, uniq, Kc, gpc):
    """Per-(bucket, gene) even-padded counts. bucket = T*8 + g."""
    cpos = np.searchsorted(uniq, ids_c)
    cchunk = cpos // Kc
    bucketid = (cchunk // 16) * 8 + (cchunk % 8)
    key = bucketid * gpc + gene_c
    cnt = np.bincount(key, minlength=32 * gpc).reshape(32, gpc)
    pad_cnt = cnt + (cnt & 1)
    return cpos, cchunk, bucketid, key, cnt, pad_cnt


def pick_cfg(snp_ids, node_seg):
    """Host pass over the indices: global compact chunk size Kc and padded
    stream length J."""
    ids, seg, gpc, gene_starts = _core_slices(snp_ids, node_seg)
    Kc = 0
    uniqs = []
    for c in range(N_CORES):
        lo, hi = gene_starts[c], gene_starts[c + 1]
        uniq = np.unique(ids[lo:hi])
        uniqs.append(uniq)
        Kc = max(Kc, -(-len(uniq) // (_NCHUNK * 16)) * 16)
    J = 0
    for c in range(N_CORES):
        lo, hi = gene_starts[c], gene_starts[c + 1]
        gene_c = seg[lo:hi] - c * gpc
        _, _, _, _, _, pad_cnt = _bucket_counts(
            ids[lo:hi], gene_c, uniqs[c], Kc, gpc
        )
        J = max(J, 2 + int(pad_cnt.sum(axis=1).max()))
    J = -(-J // 16) * 16
    return Kc, J, uniqs


def prep_inputs(cfg, snp, snp_ids, node_seg, filters, W1, uniqs):
    """Index/metadata preprocessing + zero-padding + pure layout permutation;
    all value computation happens on device."""
    import ml_dtypes

    Kc, J, gpc, gpad = cfg["Kc"], cfg["J"], cfg["gpc"], cfg["gpad"]
    nspad, d1 = cfg["nspad"], cfg["d1"]
    n_cores = cfg["n_cores"]
    ZIDX = 2 * Kc  # zero column (even; pads point here)

    ids, seg, _, gene_starts = _core_slices(snp_ids, node_seg)
    snp = np.asarray(snp, np.float32)
    filters = np.asarray(filters, np.float32)
    W1f = np.asarray(W1, np.float32)

    # mean+replicate routing: prX[m, j] = (1/8) sum_r ft[s(m)X, r, j]
    # ft row q = s*8+r; routeA: s(q) == g(m) = m//16, routeB: s(q) == 8+g(m)
    route = np.zeros((_P, 2 * _P), ml_dtypes.bfloat16)
    for m in range(_P):
        g = m // 16
        route[g * 8 : g * 8 + 8, m] = 1.0 / N_FILT
        route[(8 + g) * 8 : (8 + g) * 8 + 8, _P + m] = 1.0 / N_FILT

    sel8 = np.zeros((_P, 8), ml_dtypes.bfloat16)
    for p in range(_P):
        sel8[p, p % 8] = 1.0

    per_core = []
    for c in range(n_cores):
        lo, hi = gene_starts[c], gene_starts[c + 1]
        ids_c = ids[lo:hi]
        gene_c = seg[lo:hi] - c * gpc
        uniq = uniqs[c]
        nu = len(uniq)
        assert nu <= _NCHUNK * Kc

        # compact value tables (pure permutation of inputs)
        snp_c = np.zeros((B, _NCHUNK * Kc), np.float32)
        snp_c[:, :nu] = snp[:, uniq]
        filt_c = np.zeros((N_FILT, _NCHUNK * Kc), np.float32)
        filt_c[:, :nu] = filters[:, uniq]

        # zero-split table layout, pre-laid in DRAM (pure permutation + zero
        # padding): row p = 16g+8h+b, pass-T block cols [h*Kc : (h+1)*Kc]
        # hold snp_c[b, (16T+g+8h)*Kc + j]; the other half and the trailing
        # zero column pair stay 0
        TW = 2 * Kc + 2
        snp_perm = np.zeros((_P, _NTAB * TW), np.float32)
        sp4 = snp_perm.reshape(8, 2, 8, _NTAB * TW)  # [g, h, b, cols]
        filt_perm = np.empty((_P, _NTAB * Kc), np.float32)
        for T in range(_NTAB):
            vi = snp_c[:, 16 * T * Kc : (16 * T + 16) * Kc].reshape(B, 2, 8, Kc)
            perm = vi.transpose(2, 1, 0, 3)  # [g, h, b, j]
            for h in range(2):
                sp4[:, h, :, T * TW + h * Kc : T * TW + (h + 1) * Kc] = perm[:, h]
            fi = filt_c[:, 16 * T * Kc : (16 * T + 16) * Kc].reshape(
                N_FILT, 16, Kc
            )
            filt_perm[:, T * Kc : (T + 1) * Kc] = fi.transpose(1, 0, 2).reshape(
                _P, Kc
            )
        filt_perm_bf = filt_perm.astype(ml_dtypes.bfloat16)

        cpos, cchunk, bucketid, key, cnt, pad_cnt = _bucket_counts(
            ids_c, gene_c, uniq, Kc, gpc
        )
        clidx = cpos % Kc
        # gene-ordered per-bucket streams with even per-gene padding
        order = np.argsort(bucketid, kind="stable")  # gene order preserved
        skey = key[order]
        stbl = (clidx[order] + np.where((cchunk[order] % 16) >= 8, Kc, 0)).astype(
            np.int64
        )
        flat_cnt = cnt.reshape(-1)
        flat_pad = pad_cnt.reshape(-1)
        starts = np.zeros(32 * gpc, np.int64)  # node start per key
        np.cumsum(flat_cnt[:-1], out=starts[1:])
        offs = np.zeros(32 * gpc, np.int64)  # padded stream offset per key
        pc = flat_pad.reshape(32, gpc)
        row_off = np.cumsum(pc, axis=1)
        offs = (
            2 + np.concatenate([np.zeros((32, 1), np.int64), row_off[:, :-1]], axis=1)
        ).reshape(-1)
        rank = np.arange(len(skey), dtype=np.int64) - starts[skey]
        pos = offs[skey] + rank
        streams = np.full((32, J), ZIDX, np.int16)
        streams[bucketid[order], pos] = stbl.astype(np.int16)
        tot = 2 + pc.sum(axis=1)
        assert int(tot.max()) <= J, f"bucket {int(tot.max())} exceeds J={J}"

        # boundaries (pair units): [0, end(g0), ..., end(g_{gpc-1})], pad
        ends = ((offs.reshape(32, gpc) + pc) // 2 - 1).astype(np.int16)
        ebnd = np.zeros((32, nspad), np.int16)
        ebnd[:, 1 : gpc + 1] = ends
        ebnd[:, gpc + 1 :] = ends[:, -1:]

        gidx_all = np.concatenate(
            [_wrap16(streams[T * 8 : (T + 1) * 8]) for T in range(_NTAB)], axis=1
        )
        eidx_all = np.concatenate(
            [_wrap16(ebnd[T * 8 : (T + 1) * 8]) for T in range(_NTAB)], axis=1
        )

        w1c = np.zeros((gpad, d1), np.float32)
        w1c[:gpc] = W1f[c * gpc : (c + 1) * gpc]
        jt_ = gpad // _P
        w1perm = np.ascontiguousarray(
            w1c.reshape(jt_, _P, d1).transpose(1, 0, 2).reshape(_P, jt_ * d1)
        ).astype(ml_dtypes.bfloat16)

        per_core.append(
            dict(
                snp_perm=snp_perm, filt_perm=filt_perm_bf, sel=sel8, w1c=w1perm,
                mroute=route, gidx=gidx_all, eidx=eidx_all,
            )
        )
    return per_core


def host_tail(h1_sum, b1, g1, be1, W2, b2, g2, be2, W3, b3, g3, be3,
              Wh1, bh1, gh, beh, Wh2, bh2):
    def bn(x, g, be):
        return x * (g / np.sqrt(np.float32(1.0 + BN_EPS))) + be

    relu = lambda x: np.maximum(x, np.float32(0.0))
    h = relu(bn(h1_sum + b1, g1, be1))
    h = relu(bn(h @ W2 + b2, g2, be2))
    feat = relu(bn(h @ W3 + b3, g3, be3))
    m = relu(bn(feat[:, :15] @ Wh1 + bh1, gh, beh))
    return (m @ Wh2 + bh2).astype(np.float32)


_CACHE = {}


def kernel(snp, snp_ids, node_seg, filters, W1, b1, g1, be1, W2, b2, g2, be2,
           W3, b3, g3, be3, Wh1, bh1, gh, beh, Wh2, bh2):
    from concourse import bass_utils

    Kc, J, uniqs = pick_cfg(snp_ids, node_seg)
    cfg = make_cfg(Kc, J)

    key = ("v2", Kc, J)
    if key not in _CACHE:
        _CACHE[key] = build_program(cfg)
    nc = _CACHE[key]

    in_maps = prep_inputs(cfg, snp, snp_ids, node_seg, filters, W1, uniqs)
    res = bass_utils.run_bass_kernel_spmd(
        nc, in_maps, core_ids=list(range(cfg["n_cores"]))
    )
    h1_sum = np.zeros((B, cfg["d1"]), np.float32)
    for c in range(cfg["n_cores"]):
        h1_sum += res.results[c]["h1p"]

    f32 = lambda x: np.asarray(x, np.float32)
    return host_tail(h1_sum, f32(b1), f32(g1), f32(be1), f32(W2), f32(b2),
                     f32(g2), f32(be2), f32(W3), f32(b3), f32(g3), f32(be3),
                     f32(Wh1), f32(bh1), f32(gh), f32(beh), f32(Wh2), f32(bh2))


# revision 29
# speedup vs baseline: 1.0207x; 1.0207x over previous
"""Trainium2 Bass kernel for nn_AgeUGP_v2 (gnn_message_passing).

Reference pipeline:
  snp_h[b,n,f] = snp[b,n] * filters[f,n]
  gathered     = snp_h[:, snp_ids, :]
  per_gene     = segment_sum(gathered, node_seg)   # node_seg sorted
  sample_h     = per_gene.mean(-1)
  h1 = sample_h @ W1 ... tiny MLP tail

Algebraic collapse: the filter axis F is only averaged at the end, so
  sample_h[b,g] = sum_{i in seg g} snp[b, id_i] * fbar[id_i],
  fbar = mean(filters, axis=0).

Device strategy v2 (8 NeuronCores, genes sharded across cores):
  - Per-core SNP COMPACTION: each core's nodes reference ~197k unique SNPs
    (of 500k); the host selects and orders just those (pure permutation),
    split into 64 chunks of Kc.  4 table passes; pass T holds 16 chunks on
    128 partitions: partition p = 16g + 8h + b carries chunk 16T+g+8h,
    batch b.
  - ZERO-JUNK split tables: each partition's gather table is [2*Kc+2] with
    its chunk's values v = snp * fbar at [h*Kc : (h+1)*Kc] and ZEROS
    elsewhere (zeroed once per buffer; DMAs only rewrite data halves).
    An index in [0,Kc) reads chunk A's value on h=0 lanes and exact 0 on
    h=1 lanes (and vice versa), so the 16-lane shared-index junk vanishes
    arithmetically: A/B contributions merge into ONE gene segment.
  - fbar is produced fused on device: a bf16 host-permuted copy of filters
    is hit with 1/8-valued mean+replicate PE matmuls (routeA/routeB) whose
    PSUM output multiplies the table halves on DVE (zeros stay zero).
  - One gpsimd ap_gather per pass streams both chunks' nodes gene-ordered
    (per-gene counts padded to EVEN with pads pointing at the zero column).
    A DVE tensor_tensor_scan with data0/data1 = even/odd stride-2 views
    forms PAIR prefix sums in place (halving scan and extraction size); a
    second ap_gather extracts one prefix per gene END; one adjacent
    difference gives per-(gene,half,batch) sums; a single sel matmul per
    gene tile folds halves+lanes into sample_h [gene, batch] in PSUM.
  - PE matmul with the core's W1 shard (bf16) -> partial h1 [8, 1024];
    host sums the 8 partials and runs the tiny MLP tail (0.01% of FLOPs).
Emission is software-pipelined (gather p+1 ahead of pass-p tail; tables
manually double-buffered so the zero halves persist across passes).
"""

import numpy as np

B = 8
N_SNPS = 500000
N_NODES = 2000000
N_GENES = 20000
N_FILT = 8
N_CORES = 8
BN_EPS = 1e-5

_P = 128
_NCHUNK = 64  # compact SNP chunks per core
_NTAB = 4  # table passes
_EPAD = 16


def make_cfg(Kc, J, n_genes=N_GENES, n_cores=N_CORES, d1=1024):
    gpc = n_genes // n_cores
    jt = -(-gpc // _P)
    gpad = jt * _P
    ns = gpc + 1  # boundaries: dummy zero + one end per gene
    nspad = -(-ns // _EPAD) * _EPAD
    assert J % 16 == 0 and J % 4 == 0
    assert 2 * Kc + 2 <= 2**15, "gather table exceeds num_elems limit"
    assert J <= 32752, "stream length exceeds int16 index range"
    return dict(
        Kc=Kc, J=J, gpc=gpc, gpad=gpad, jt=jt, d1=d1, ns=ns, nspad=nspad,
        n_cores=n_cores,
    )


# ---------------------------------------------------------------- device program
def build_program(cfg):
    import concourse.bass as bass
    import concourse.bacc as bacc
    import concourse.mybir as mybir
    import concourse.tile as tile

    fp32 = mybir.dt.float32
    bf16 = mybir.dt.bfloat16
    i16 = mybir.dt.int16

    Kc, J = cfg["Kc"], cfg["J"]
    jt, d1 = cfg["jt"], cfg["d1"]
    gpc, gpad, nspad = cfg["gpc"], cfg["gpad"], cfg["nspad"]
    TW = 2 * Kc + 2  # table width: [A-half | B-half | zero col pair]
    JH = J // 2

    nc = bacc.Bacc(
        "TRN2", target_bir_lowering=False, debug=False, num_devices=cfg["n_cores"]
    )

    snp_in = nc.dram_tensor("snp_perm", [_P, _NTAB * TW], fp32, kind="ExternalInput")
    filt_in = nc.dram_tensor("filt_perm", [_P, _NTAB * Kc], bf16, kind="ExternalInput")
    gidx_in = nc.dram_tensor("gidx", [_P, _NTAB * (J // 16)], i16, kind="ExternalInput")
    eidx_in = nc.dram_tensor(
        "eidx", [_P, _NTAB * (nspad // 16)], i16, kind="ExternalInput"
    )
    sel_in = nc.dram_tensor("sel", [_P, 8], bf16, kind="ExternalInput")
    route_in = nc.dram_tensor("mroute", [_P, 2 * _P], bf16, kind="ExternalInput")
    w1_in = nc.dram_tensor("w1c", [_P, jt * d1], bf16, kind="ExternalInput")
    h1_out = nc.dram_tensor("h1p", [B, d1], fp32, kind="ExternalOutput")

    rc = Kc // 4  # route/mul block width (2-bank PSUM tiles)
    assert rc * 4 == Kc and rc * 4 <= 4096

    with tile.TileContext(nc) as tc:
        with (
            tc.tile_pool(name="per", bufs=1) as perpool,
            tc.tile_pool(name="tab", bufs=2) as tabpool,
            tc.tile_pool(name="gs", bufs=2) as gspool,
            tc.tile_pool(name="ft", bufs=2) as ftpool,
            tc.tile_pool(name="ex", bufs=1) as expool,
            tc.tile_pool(name="w1", bufs=5) as w1pool,
            tc.tile_pool(name="ps", bufs=2, space="PSUM") as pspool,
            tc.tile_pool(name="psw", bufs=1, space="PSUM") as pswpool,
            tc.tile_pool(name="psh", bufs=2, space="PSUM") as pshpool,
        ):
            DMAQ, DVEQ, POOLQ, PEQ = [], [], [], []

            route = perpool.tile([_P, 2 * _P], bf16, tag="route")
            route_d = nc.sync.dma_start(route[:], route_in.ap())
            sel8 = perpool.tile([_P, 8], bf16, tag="sel8")
            sel_d = nc.sync.dma_start(sel8[:], sel_in.ap())

            # sample_h accumulator [gene-tile, (t, b)]
            sh = perpool.tile([_P, jt * B], fp32, tag="sh")
            DVEQ.append(nc.vector.memset(sh[:], 0.0))
            # dd holds per-(lane,gene) sums; pad cols stay zero forever
            dd = perpool.tile([_P, gpad], bf16, tag="dd")
            DVEQ.append(nc.vector.memset(dd[:], 0.0))

            from concourse.instruction_name_ordered_set import (
                InstructionNameOrderedSet,
            )

            def pin(later, earlier):
                """Same-engine order pin (no runtime semaphore)."""
                s = InstructionNameOrderedSet()
                s.add(earlier.ins.name)
                later.ins.add_nosync_dependencies_from(s)

            vtabs = {}

            def emit_table(T):
                # DRAM rows carry the zero-split layout already (data half +
                # zero half per partition parity): two wide DMAs per pass,
                # the zero columns arrive as part of the transfer
                vtab = tabpool.tile([_P, TW], fp32, tag="vtab", name=f"vtab{T}")
                ft = ftpool.tile([_P, Kc], bf16, tag="ftl", name=f"ftl{T}")
                dmas = [
                    nc.sync.dma_start(
                        ft[:], filt_in.ap()[:, T * Kc : (T + 1) * Kc]
                    ),
                    nc.sync.dma_start(
                        vtab[:, 0:Kc], snp_in.ap()[:, T * TW : T * TW + Kc]
                    ),
                    nc.sync.dma_start(
                        vtab[:, Kc:TW], snp_in.ap()[:, T * TW + Kc : (T + 1) * TW]
                    ),
                ]
                muls, prs = [], []
                for half in range(2):
                    for blk in range(4):
                        pr = pspool.tile([_P, rc], fp32, tag="pr", name="pr")
                        prs.append(
                            nc.tensor.matmul(
                                pr[:],
                                route[:, half * _P : (half + 1) * _P],
                                ft[:, blk * rc : (blk + 1) * rc],
                                start=True, stop=True,
                            )
                        )
                        ks = slice(half * Kc + blk * rc, half * Kc + (blk + 1) * rc)
                        muls.append(
                            nc.vector.tensor_mul(vtab[:, ks], vtab[:, ks], pr[:])
                        )
                vtabs[T] = vtab
                return dict(dmas=dmas, muls=muls, prs=prs)

            # index streams prefetched once (each pass's stream is its own
            # tile: ap_gather idx APs must start at a tile base)
            gidx_t, eidx_t = {}, {}

            def prefetch_idx():
                dmas = []
                for p in range(_NTAB):
                    g = perpool.tile([_P, J // 16], i16, tag=f"gidx{p}")
                    dmas.append(
                        nc.sync.dma_start(
                            g[:],
                            gidx_in.ap()[:, p * (J // 16) : (p + 1) * (J // 16)],
                        )
                    )
                    gidx_t[p] = g
                    e = perpool.tile([_P, nspad // 16], i16, tag=f"eidx{p}")
                    dmas.append(
                        nc.sync.dma_start(
                            e[:],
                            eidx_in.ap()[
                                :, p * (nspad // 16) : (p + 1) * (nspad // 16)
                            ],
                        )
                    )
                    eidx_t[p] = e
                return dmas

            def emit_gather(p):
                gidx = gidx_t[p]
                gout = gspool.tile([_P, J], fp32, tag="gout", name=f"gout{p}")
                g1 = nc.gpsimd.ap_gather(
                    gout[:], vtabs.pop(p)[:], gidx[:],
                    channels=_P, num_elems=TW, d=1, num_idxs=J,
                )
                return gout, g1

            def emit_scan_extract(p, gout):
                # pair prefix scan, in place into the first half (writes
                # trail the stride-2 reads)
                gall = gout[:]
                even = bass.AP(gall.tensor, gall.offset, [gall.ap[0], [2, JH]])
                godd = gout[:, 1:]
                odd = bass.AP(godd.tensor, godd.offset, [godd.ap[0], [2, JH]])
                sc = nc.vector.tensor_tensor_scan(
                    gout[:, :JH], even, odd, 0.0,
                    op0=mybir.AluOpType.add, op1=mybir.AluOpType.add,
                )
                eidx = eidx_t[p]
                ex = expool.tile([_P, nspad], fp32, tag="ex", name=f"ex{p}")
                g2 = nc.gpsimd.ap_gather(
                    ex[:], gout[:, :JH], eidx[:],
                    channels=_P, num_elems=JH, d=1, num_idxs=nspad,
                )
                return sc, g2, ex

            def emit_reduce(p, ex):
                sub = nc.vector.tensor_sub(
                    dd[:, :gpc], ex[:, 1 : gpc + 1], ex[:, :gpc]
                )
                pst = pshpool.tile([_P, jt * B], fp32, tag="pst", name="pst")
                mms = []
                for t in range(jt):
                    mms.append(
                        nc.tensor.matmul(
                            pst[:, t * B : (t + 1) * B],
                            dd[:, t * _P : (t + 1) * _P],
                            sel8[:],
                            start=True, stop=True,
                        )
                    )
                add = nc.vector.tensor_add(sh[:], sh[:], pst[:])
                return sub, add, mms

            # Emission with a fully pinned per-engine static schedule (the
            # tile scheduler's internal timing model mispredicts DMA
            # completion and otherwise serializes the pipeline):
            #   Pool: g1(0) g1(1) g2(0) g1(2) g2(1) g1(3) g2(2) g2(3)
            #   DVE : muls(0) muls(1) scan0 muls2 scan1 muls3 dd0 scan2
            #         dd1 scan3 dd2 dd3 shb copies
            #   DMA : tables in pass order, idx after table0, W1 after B3
            wgrp = 4 if jt % 4 == 0 else 1  # K-tiles per W1 load
            tabs = {0: emit_table(0)}
            idx_dmas = prefetch_idx()
            tabs[1] = emit_table(1)
            w1ts, w1dmas = [], []
            for jg in range(jt // wgrp):
                w1t = w1pool.tile([_P, wgrp * d1], bf16, tag="w1t", name=f"w1t{jg}")
                w1dmas.append(
                    nc.sync.dma_start(
                        w1t[:], w1_in.ap()[:, jg * wgrp * d1 : (jg + 1) * wgrp * d1]
                    )
                )
                w1ts.append(w1t)
            gouts, g1i, scans, g2i, exs = {}, {}, {}, {}, {}
            reds = {}
            gouts[0], g1i[0] = emit_gather(0)
            for p in range(_NTAB):
                if p + 1 < _NTAB:
                    gouts[p + 1], g1i[p + 1] = emit_gather(p + 1)
                scans[p], g2i[p], exs[p] = emit_scan_extract(p, gouts.pop(p))
                if p + 2 < _NTAB:
                    tabs[p + 2] = emit_table(p + 2)
                if p >= 1:
                    reds[p - 1] = emit_reduce(p - 1, exs.pop(p - 1))
            reds[_NTAB - 1] = emit_reduce(_NTAB - 1, exs.pop(_NTAB - 1))

            shb = perpool.tile([_P, jt * B], bf16, tag="shb")
            shb_c = nc.vector.tensor_copy(shb[:], sh[:])

            # ---- W1 matmul: accumulate over jt K-tiles --------------------
            n_half = min(512, d1)
            n_banks = -(-d1 // n_half)
            pss = []
            for nb in range(n_banks):
                pst = pswpool.tile([_P, n_half], fp32, tag=f"ps{nb}", name=f"ps{nb}")
                pss.append(pst)
            w1mms = []
            for jg in range(jt // wgrp):
                w1t = w1ts[jg]
                for jl in range(wgrp):
                    j = jg * wgrp + jl
                    lhsT = shb[:, j * B : (j + 1) * B]
                    for nb in range(n_banks):
                        w1mms.append(
                            nc.tensor.matmul(
                                pss[nb][:B, :],
                                lhsT,
                                w1t[:, jl * d1 + nb * n_half : jl * d1 + (nb + 1) * n_half],
                                start=(j == 0),
                                stop=(j == jt - 1),
                            )
                        )

            h1 = perpool.tile([B, d1], fp32, tag="h1")
            h1copies = []
            for nb in range(n_banks):
                h1copies.append(
                    nc.vector.tensor_copy(
                        h1[:, nb * n_half : (nb + 1) * n_half], pss[nb][:B, :]
                    )
                )
            DMAQ_out = nc.sync.dma_start(h1_out.ap(), h1[:])

            # ---------------- static order pins ----------------
            def chain(seq):
                for a, b in zip(seq, seq[1:]):
                    pin(b, a)

            # DMA: table 0 first (route/sel slot in after its A half so the
            # first multiplies start as early as possible), then table 1,
            # index streams, remaining tables, W1, output
            ft0, A0, B0 = tabs[0]["dmas"]
            DMAQ += [ft0, A0, route_d, sel_d, B0]
            DMAQ += tabs[1]["dmas"] + idx_dmas
            DMAQ += tabs[2]["dmas"] + tabs[3]["dmas"] + w1dmas + [DMAQ_out]
            chain(DMAQ)
            # Pool: strict alternation, next gather ahead of extraction
            POOLQ += [g1i[0], g1i[1], g2i[0], g1i[2], g2i[1], g1i[3],
                      g2i[2], g2i[3]]
            chain(POOLQ)
            # DVE: table muls for p+2 between scan(p) and scan(p+1);
            # reduces as soon as their extraction lands
            DVEQ += tabs[0]["muls"] + tabs[1]["muls"]
            DVEQ += [scans[0]] + tabs[2]["muls"] + [scans[1]] + tabs[3]["muls"]
            DVEQ += [reds[0][0], reds[0][1], scans[2], reds[1][0], reds[1][1]]
            DVEQ += [scans[3], reds[2][0], reds[2][1], reds[3][0], reds[3][1]]
            DVEQ += [shb_c] + h1copies
            chain(DVEQ)
            # PE: route matmuls in pass order, then sel matmuls, then W1
            PEQ += tabs[0]["prs"] + tabs[1]["prs"] + tabs[2]["prs"]
            PEQ += tabs[3]["prs"]
            for p in range(_NTAB):
                PEQ += reds[p][2]
            PEQ += w1mms
            chain(PEQ)

    nc.compile()
    return nc


# ---------------------------------------------------------------- host side
def _wrap16(streams):
    """[8, J] per-group streams -> [128, J//16] wrapped-16 layout."""
    ngrp, J = streams.shape
    assert ngrp == 8 and J % 16 == 0
    out = np.zeros((_P, J // 16), streams.dtype)
    for g in range(8):
        out[g * 16 : (g + 1) * 16, :] = streams[g].reshape(J // 16, 16).T
    return out


def _core_slices(snp_ids, node_seg):
    ids = np.asarray(snp_ids).astype(np.int64)
    seg = np.asarray(node_seg).astype(np.int64)
    gpc = N_GENES // N_CORES
    gene_starts = np.searchsorted(seg, np.arange(0, N_GENES + 1, gpc))
    return ids, seg, gpc, gene_starts


def _bucket_counts(ids_c, gene_c, uniq, Kc, gpc):
    """Per-(bucket, gene) even-padded counts. bucket = T*8 + g."""
    cpos = np.searchsorted(uniq, ids_c)
    cchunk = cpos // Kc
    bucketid = (cchunk // 16) * 8 + (cchunk % 8)
    key = bucketid * gpc + gene_c
    cnt = np.bincount(key, minlength=32 * gpc).reshape(32, gpc)
    pad_cnt = cnt + (cnt & 1)
    return cpos, cchunk, bucketid, key, cnt, pad_cnt


def pick_cfg(snp_ids, node_seg):
    """Host pass over the indices: global compact chunk size Kc and padded
    stream length J."""
    ids, seg, gpc, gene_starts = _core_slices(snp_ids, node_seg)
    Kc = 0
    uniqs = []
    for c in range(N_CORES):
        lo, hi = gene_starts[c], gene_starts[c + 1]
        uniq = np.unique(ids[lo:hi])
        uniqs.append(uniq)
        Kc = max(Kc, -(-len(uniq) // (_NCHUNK * 16)) * 16)
    J = 0
    for c in range(N_CORES):
        lo, hi = gene_starts[c], gene_starts[c + 1]
        gene_c = seg[lo:hi] - c * gpc
        _, _, _, _, _, pad_cnt = _bucket_counts(
            ids[lo:hi], gene_c, uniqs[c], Kc, gpc
        )
        J = max(J, 2 + int(pad_cnt.sum(axis=1).max()))
    J = -(-J // 16) * 16
    return Kc, J, uniqs


def prep_inputs(cfg, snp, snp_ids, node_seg, filters, W1, uniqs):
    """Index/metadata preprocessing + zero-padding + pure layout permutation;
    all value computation happens on device."""
    import ml_dtypes

    Kc, J, gpc, gpad = cfg["Kc"], cfg["J"], cfg["gpc"], cfg["gpad"]
    nspad, d1 = cfg["nspad"], cfg["d1"]
    n_cores = cfg["n_cores"]
    ZIDX = 2 * Kc  # zero column (even; pads point here)

    ids, seg, _, gene_starts = _core_slices(snp_ids, node_seg)
    snp = np.asarray(snp, np.float32)
    filters = np.asarray(filters, np.float32)
    W1f = np.asarray(W1, np.float32)

    # mean+replicate routing: prX[m, j] = (1/8) sum_r ft[s(m)X, r, j]
    # ft row q = s*8+r; routeA: s(q) == g(m) = m//16, routeB: s(q) == 8+g(m)
    route = np.zeros((_P, 2 * _P), ml_dtypes.bfloat16)
    for m in range(_P):
        g = m // 16
        route[g * 8 : g * 8 + 8, m] = 1.0 / N_FILT
        route[(8 + g) * 8 : (8 + g) * 8 + 8, _P + m] = 1.0 / N_FILT

    sel8 = np.zeros((_P, 8), ml_dtypes.bfloat16)
    for p in range(_P):
        sel8[p, p % 8] = 1.0

    per_core = []
    for c in range(n_cores):
        lo, hi = gene_starts[c], gene_starts[c + 1]
        ids_c = ids[lo:hi]
        gene_c = seg[lo:hi] - c * gpc
        uniq = uniqs[c]
        nu = len(uniq)
        assert nu <= _NCHUNK * Kc

        # compact value tables (pure permutation of inputs)
        snp_c = np.zeros((B, _NCHUNK * Kc), np.float32)
        snp_c[:, :nu] = snp[:, uniq]
        filt_c = np.zeros((N_FILT, _NCHUNK * Kc), np.float32)
        filt_c[:, :nu] = filters[:, uniq]

        # zero-split table layout, pre-laid in DRAM (pure permutation + zero
        # padding): row p = 16g+8h+b, pass-T block cols [h*Kc : (h+1)*Kc]
        # hold snp_c[b, (16T+g+8h)*Kc + j]; the other half and the trailing
        # zero column pair stay 0
        TW = 2 * Kc + 2
        snp_perm = np.zeros((_P, _NTAB * TW), np.float32)
        sp4 = snp_perm.reshape(8, 2, 8, _NTAB * TW)  # [g, h, b, cols]
        filt_perm = np.empty((_P, _NTAB * Kc), np.float32)
        for T in range(_NTAB):
            vi = snp_c[:, 16 * T * Kc : (16 * T + 16) * Kc].reshape(B, 2, 8, Kc)
            perm = vi.transpose(2, 1, 0, 3)  # [g, h, b, j]
            for h in range(2):
                sp4[:, h, :, T * TW + h * Kc : T * TW + (h + 1) * Kc] = perm[:, h]
            fi = filt_c[:, 16 * T * Kc : (16 * T + 16) * Kc].reshape(
                N_FILT, 16, Kc
            )
            filt_perm[:, T * Kc : (T + 1) * Kc] = fi.transpose(1, 0, 2).reshape(
                _P, Kc
            )
        filt_perm_bf = filt_perm.astype(ml_dtypes.bfloat16)

        cpos, cchunk, bucketid, key, cnt, pad_cnt = _bucket_counts(
            ids_c, gene_c, uniq, Kc, gpc
        )
        clidx = cpos % Kc
        # gene-ordered per-bucket streams with even per-gene padding
        order = np.argsort(bucketid, kind="stable")  # gene order preserved
        skey = key[order]
        stbl = (clidx[order] + np.where((cchunk[order] % 16) >= 8, Kc, 0)).astype(
            np.int64
        )
        flat_cnt = cnt.reshape(-1)
        flat_pad = pad_cnt.reshape(-1)
        starts = np.zeros(32 * gpc, np.int64)  # node start per key
        np.cumsum(flat_cnt[:-1], out=starts[1:])
        offs = np.zeros(32 * gpc, np.int64)  # padded stream offset per key
        pc = flat_pad.reshape(32, gpc)
        row_off = np.cumsum(pc, axis=1)
        offs = (
            2 + np.concatenate([np.zeros((32, 1), np.int64), row_off[:, :-1]], axis=1)
        ).reshape(-1)
        rank = np.arange(len(skey), dtype=np.int64) - starts[skey]
        pos = offs[skey] + rank
        streams = np.full((32, J), ZIDX, np.int16)
        streams[bucketid[order], pos] = stbl.astype(np.int16)
        tot = 2 + pc.sum(axis=1)
        assert int(tot.max()) <= J, f"bucket {int(tot.max())} exceeds J={J}"

        # boundaries (pair units): [0, end(g0), ..., end(g_{gpc-1})], pad
        ends = ((offs.reshape(32, gpc) + pc) // 2 - 1).astype(np.int16)
        ebnd = np.zeros((32, nspad), np.int16)
        ebnd[:, 1 : gpc + 1] = ends
        ebnd[:, gpc + 1 :] = ends[:, -1:]

        gidx_all = np.concatenate(
            [_wrap16(streams[T * 8 : (T + 1) * 8]) for T in range(_NTAB)], axis=1
        )
        eidx_all = np.concatenate(
            [_wrap16(ebnd[T * 8 : (T + 1) * 8]) for T in range(_NTAB)], axis=1
        )

        w1c = np.zeros((gpad, d1), np.float32)
        w1c[:gpc] = W1f[c * gpc : (c + 1) * gpc]
        jt_ = gpad // _P
        w1perm = np.ascontiguousarray(
            w1c.reshape(jt_, _P, d1).transpose(1, 0, 2).reshape(_P, jt_ * d1)
        ).astype(ml_dtypes.bfloat16)

        per_core.append(
            dict(
                snp_perm=snp_perm, filt_perm=filt_perm_bf, sel=sel8, w1c=w1perm,
                mroute=route, gidx=gidx_all, eidx=eidx_all,
            )
        )
    return per_core


def host_tail(h1_sum, b1, g1, be1, W2, b2, g2, be2, W3, b3, g3, be3,
              Wh1, bh1, gh, beh, Wh2, bh2):
    def bn(x, g, be):
        return x * (g / np.sqrt(np.float32(1.0 + BN_EPS))) + be

    relu = lambda x: np.maximum(x, np.float32(0.0))
    h = relu(bn(h1_sum + b1, g1, be1))
    h = relu(bn(h @ W2 + b2, g2, be2))
    feat = relu(bn(h @ W3 + b3, g3, be3))
    m = relu(bn(feat[:, :15] @ Wh1 + bh1, gh, beh))
    return (m @ Wh2 + bh2).astype(np.float32)


_CACHE = {}


def kernel(snp, snp_ids, node_seg, filters, W1, b1, g1, be1, W2, b2, g2, be2,
           W3, b3, g3, be3, Wh1, bh1, gh, beh, Wh2, bh2):
    from concourse import bass_utils

    Kc, J, uniqs = pick_cfg(snp_ids, node_seg)
    cfg = make_cfg(Kc, J)

    key = ("v2", Kc, J)
    if key not in _CACHE:
        _CACHE[key] = build_program(cfg)
    nc = _CACHE[key]

    in_maps = prep_inputs(cfg, snp, snp_ids, node_seg, filters, W1, uniqs)
    res = bass_utils.run_bass_kernel_spmd(
        nc, in_maps, core_ids=list(range(cfg["n_cores"]))
    )
    h1_sum = np.zeros((B, cfg["d1"]), np.float32)
    for c in range(cfg["n_cores"]):
        h1_sum += res.results[c]["h1p"]

    f32 = lambda x: np.asarray(x, np.float32)
    return host_tail(h1_sum, f32(b1), f32(g1), f32(be1), f32(W2), f32(b2),
                     f32(g2), f32(be2), f32(W3), f32(b3), f32(g3), f32(be3),
                     f32(Wh1), f32(bh1), f32(gh), f32(beh), f32(Wh2), f32(bh2))


# revision 30
# speedup vs baseline: 1.0770x; 1.0551x over previous
"""Trainium2 Bass kernel for nn_AgeUGP_v2 (gnn_message_passing).

Reference pipeline:
  snp_h[b,n,f] = snp[b,n] * filters[f,n]
  gathered     = snp_h[:, snp_ids, :]
  per_gene     = segment_sum(gathered, node_seg)   # node_seg sorted
  sample_h     = per_gene.mean(-1)
  h1 = sample_h @ W1 ... tiny MLP tail

Algebraic collapse: the filter axis F is only averaged at the end, so
  sample_h[b,g] = sum_{i in seg g} snp[b, id_i] * fbar[id_i],
  fbar = mean(filters, axis=0).

Device strategy v2 (8 NeuronCores, genes sharded across cores):
  - Per-core SNP COMPACTION: each core's nodes reference ~197k unique SNPs
    (of 500k); the host selects and orders just those (pure permutation),
    split into 64 chunks of Kc.  4 table passes; pass T holds 16 chunks on
    128 partitions: partition p = 16g + 8h + b carries chunk 16T+g+8h,
    batch b.
  - ZERO-JUNK split tables: each partition's gather table is [2*Kc+2] with
    its chunk's values v = snp * fbar at [h*Kc : (h+1)*Kc] and ZEROS
    elsewhere (zeroed once per buffer; DMAs only rewrite data halves).
    An index in [0,Kc) reads chunk A's value on h=0 lanes and exact 0 on
    h=1 lanes (and vice versa), so the 16-lane shared-index junk vanishes
    arithmetically: A/B contributions merge into ONE gene segment.
  - fbar is produced fused on device: a bf16 host-permuted copy of filters
    is hit with 1/8-valued mean+replicate PE matmuls (routeA/routeB) whose
    PSUM output multiplies the table halves on DVE (zeros stay zero).
  - One gpsimd ap_gather per pass streams both chunks' nodes gene-ordered
    (per-gene counts padded to EVEN with pads pointing at the zero column).
    A DVE tensor_tensor_scan with data0/data1 = even/odd stride-2 views
    forms PAIR prefix sums in place (halving scan and extraction size); a
    second ap_gather extracts one prefix per gene END; one adjacent
    difference gives per-(gene,half,batch) sums; a single sel matmul per
    gene tile folds halves+lanes into sample_h [gene, batch] in PSUM.
  - PE matmul with the core's W1 shard (bf16) -> partial h1 [8, 1024];
    host sums the 8 partials and runs the tiny MLP tail (0.01% of FLOPs).
Emission is software-pipelined (gather p+1 ahead of pass-p tail; tables
manually double-buffered so the zero halves persist across passes).
"""

import numpy as np

B = 8
N_SNPS = 500000
N_NODES = 2000000
N_GENES = 20000
N_FILT = 8
N_CORES = 8
BN_EPS = 1e-5

_P = 128
_NCHUNK = 64  # compact SNP chunks per core
_NTAB = 4  # table passes
_EPAD = 16


def make_cfg(Kc, J, n_genes=N_GENES, n_cores=N_CORES, d1=1024):
    gpc = n_genes // n_cores
    jt = -(-gpc // _P)
    gpad = jt * _P
    ns = gpc + 1  # boundaries: dummy zero + one end per gene
    nspad = -(-ns // _EPAD) * _EPAD
    assert J % 16 == 0 and J % 4 == 0
    assert 2 * Kc + 2 <= 2**15, "gather table exceeds num_elems limit"
    assert J <= 32752, "stream length exceeds int16 index range"
    return dict(
        Kc=Kc, J=J, gpc=gpc, gpad=gpad, jt=jt, d1=d1, ns=ns, nspad=nspad,
        n_cores=n_cores,
    )


# ---------------------------------------------------------------- device program
def build_program(cfg):
    import concourse.bass as bass
    import concourse.bacc as bacc
    import concourse.mybir as mybir
    import concourse.tile as tile

    fp32 = mybir.dt.float32
    bf16 = mybir.dt.bfloat16
    i16 = mybir.dt.int16

    Kc, J = cfg["Kc"], cfg["J"]
    jt, d1 = cfg["jt"], cfg["d1"]
    gpc, gpad, nspad = cfg["gpc"], cfg["gpad"], cfg["nspad"]
    TW = 2 * Kc + 2  # table width: [A-half | B-half | zero col pair]
    JH = J // 2

    nc = bacc.Bacc(
        "TRN2", target_bir_lowering=False, debug=False, num_devices=cfg["n_cores"]
    )

    snp_in = nc.dram_tensor("snp_perm", [_P, _NTAB * TW], fp32, kind="ExternalInput")
    filt_in = nc.dram_tensor("filt_perm", [_P, _NTAB * Kc], bf16, kind="ExternalInput")
    gidx_in = nc.dram_tensor("gidx", [_P, _NTAB * (J // 16)], i16, kind="ExternalInput")
    eidx_in = nc.dram_tensor(
        "eidx", [_P, _NTAB * (nspad // 16)], i16, kind="ExternalInput"
    )
    sel_in = nc.dram_tensor("sel", [_P, 8], bf16, kind="ExternalInput")
    route_in = nc.dram_tensor("mroute", [_P, 2 * _P], bf16, kind="ExternalInput")
    w1_in = nc.dram_tensor("w1c", [_P, jt * d1], bf16, kind="ExternalInput")
    h1_out = nc.dram_tensor("h1p", [B, d1], fp32, kind="ExternalOutput")

    rc = Kc // 4  # route/mul block width (2-bank PSUM tiles)
    assert rc * 4 == Kc and rc * 4 <= 4096

    with tile.TileContext(nc) as tc:
        with (
            tc.tile_pool(name="per", bufs=1) as perpool,
            tc.tile_pool(name="tab", bufs=2) as tabpool,
            tc.tile_pool(name="gs", bufs=2) as gspool,
            tc.tile_pool(name="ft", bufs=2) as ftpool,
            tc.tile_pool(name="ex", bufs=1) as expool,
            tc.tile_pool(name="w1", bufs=5) as w1pool,
            tc.tile_pool(name="ps", bufs=2, space="PSUM") as pspool,
            tc.tile_pool(name="psw", bufs=1, space="PSUM") as pswpool,
            tc.tile_pool(name="psh", bufs=2, space="PSUM") as pshpool,
        ):
            DMAQ, DVEQ, POOLQ, PEQ = [], [], [], []

            route = perpool.tile([_P, 2 * _P], bf16, tag="route")
            route_d = nc.sync.dma_start(route[:], route_in.ap())
            sel8 = perpool.tile([_P, 8], bf16, tag="sel8")
            sel_d = nc.sync.dma_start(sel8[:], sel_in.ap())

            # sample_h accumulator [gene-tile, (t, b)]
            sh = perpool.tile([_P, jt * B], fp32, tag="sh")
            DVEQ.append(nc.vector.memset(sh[:], 0.0))
            # dd holds per-(lane,gene) sums; pad cols stay zero forever
            dd = perpool.tile([_P, gpad], bf16, tag="dd")
            DVEQ.append(nc.vector.memset(dd[:], 0.0))

            from concourse.instruction_name_ordered_set import (
                InstructionNameOrderedSet,
            )

            def pin(later, earlier):
                """Same-engine order pin (no runtime semaphore)."""
                s = InstructionNameOrderedSet()
                s.add(earlier.ins.name)
                later.ins.add_nosync_dependencies_from(s)

            vtabs = {}

            def emit_table(T):
                # DRAM rows carry the zero-split layout already (data half +
                # zero half per partition parity): two wide DMAs per pass,
                # the zero columns arrive as part of the transfer
                vtab = tabpool.tile([_P, TW], fp32, tag="vtab", name=f"vtab{T}")
                ft = ftpool.tile([_P, Kc], bf16, tag="ftl", name=f"ftl{T}")
                dmas = [
                    nc.sync.dma_start(
                        ft[:], filt_in.ap()[:, T * Kc : (T + 1) * Kc]
                    ),
                    nc.sync.dma_start(
                        vtab[:, 0:Kc], snp_in.ap()[:, T * TW : T * TW + Kc]
                    ),
                    nc.sync.dma_start(
                        vtab[:, Kc:TW], snp_in.ap()[:, T * TW + Kc : (T + 1) * TW]
                    ),
                ]
                muls, prs = [], []
                for half in range(2):
                    for blk in range(4):
                        pr = pspool.tile([_P, rc], fp32, tag="pr", name="pr")
                        prs.append(
                            nc.tensor.matmul(
                                pr[:],
                                route[:, half * _P : (half + 1) * _P],
                                ft[:, blk * rc : (blk + 1) * rc],
                                start=True, stop=True,
                            )
                        )
                        ks = slice(half * Kc + blk * rc, half * Kc + (blk + 1) * rc)
                        muls.append(
                            nc.vector.tensor_mul(vtab[:, ks], vtab[:, ks], pr[:])
                        )
                vtabs[T] = vtab
                return dict(dmas=dmas, muls=muls, prs=prs)

            # index streams prefetched once (each pass's stream is its own
            # tile: ap_gather idx APs must start at a tile base)
            gidx_t, eidx_t = {}, {}

            def prefetch_idx():
                dmas = []
                for p in range(_NTAB):
                    g = perpool.tile([_P, J // 16], i16, tag=f"gidx{p}")
                    dmas.append(
                        nc.sync.dma_start(
                            g[:],
                            gidx_in.ap()[:, p * (J // 16) : (p + 1) * (J // 16)],
                        )
                    )
                    gidx_t[p] = g
                    e = perpool.tile([_P, nspad // 16], i16, tag=f"eidx{p}")
                    dmas.append(
                        nc.sync.dma_start(
                            e[:],
                            eidx_in.ap()[
                                :, p * (nspad // 16) : (p + 1) * (nspad // 16)
                            ],
                        )
                    )
                    eidx_t[p] = e
                return dmas

            def emit_gather(p):
                gidx = gidx_t[p]
                gout = gspool.tile([_P, J], fp32, tag="gout", name=f"gout{p}")
                g1 = nc.gpsimd.ap_gather(
                    gout[:], vtabs.pop(p)[:], gidx[:],
                    channels=_P, num_elems=TW, d=1, num_idxs=J,
                )
                return gout, g1

            def emit_scan_extract(p, gout):
                # pair prefix scan, in place into the first half (writes
                # trail the stride-2 reads)
                gall = gout[:]
                even = bass.AP(gall.tensor, gall.offset, [gall.ap[0], [2, JH]])
                godd = gout[:, 1:]
                odd = bass.AP(godd.tensor, godd.offset, [godd.ap[0], [2, JH]])
                sc = nc.vector.tensor_tensor_scan(
                    gout[:, :JH], even, odd, 0.0,
                    op0=mybir.AluOpType.add, op1=mybir.AluOpType.add,
                )
                eidx = eidx_t[p]
                ex = expool.tile([_P, nspad], fp32, tag="ex", name=f"ex{p}")
                g2 = nc.gpsimd.ap_gather(
                    ex[:], gout[:, :JH], eidx[:],
                    channels=_P, num_elems=JH, d=1, num_idxs=nspad,
                )
                return sc, g2, ex

            def emit_reduce(p, ex):
                sub = nc.vector.tensor_sub(
                    dd[:, :gpc], ex[:, 1 : gpc + 1], ex[:, :gpc]
                )
                pst = pshpool.tile([_P, jt * B], fp32, tag="pst", name="pst")
                mms = []
                for t in range(jt):
                    mms.append(
                        nc.tensor.matmul(
                            pst[:, t * B : (t + 1) * B],
                            dd[:, t * _P : (t + 1) * _P],
                            sel8[:],
                            start=True, stop=True,
                        )
                    )
                add = nc.vector.tensor_add(sh[:], sh[:], pst[:])
                return sub, add, mms

            # Emission with a fully pinned per-engine static schedule (the
            # tile scheduler's internal timing model mispredicts DMA
            # completion and otherwise serializes the pipeline):
            #   Pool: g1(0) g1(1) g2(0) g1(2) g2(1) g1(3) g2(2) g2(3)
            #   DVE : muls(0) muls(1) scan0 muls2 scan1 muls3 dd0 scan2
            #         dd1 scan3 dd2 dd3 shb copies
            #   DMA : tables in pass order, idx after table0, W1 after B3
            wgrp = 4 if jt % 4 == 0 else 1  # K-tiles per W1 load
            tabs = {0: emit_table(0)}
            idx_dmas = prefetch_idx()
            tabs[1] = emit_table(1)
            w1ts, w1dmas = [], []
            for jg in range(jt // wgrp):
                w1t = w1pool.tile([_P, wgrp * d1], bf16, tag="w1t", name=f"w1t{jg}")
                w1dmas.append(
                    nc.sync.dma_start(
                        w1t[:], w1_in.ap()[:, jg * wgrp * d1 : (jg + 1) * wgrp * d1]
                    )
                )
                w1ts.append(w1t)
            gouts, g1i, scans, g2i, exs = {}, {}, {}, {}, {}
            reds = {}
            gouts[0], g1i[0] = emit_gather(0)
            for p in range(_NTAB):
                if p + 1 < _NTAB:
                    gouts[p + 1], g1i[p + 1] = emit_gather(p + 1)
                scans[p], g2i[p], exs[p] = emit_scan_extract(p, gouts.pop(p))
                if p + 2 < _NTAB:
                    tabs[p + 2] = emit_table(p + 2)
                if p >= 1:
                    reds[p - 1] = emit_reduce(p - 1, exs.pop(p - 1))
            reds[_NTAB - 1] = emit_reduce(_NTAB - 1, exs.pop(_NTAB - 1))

            shb = perpool.tile([_P, jt * B], bf16, tag="shb")
            shb_c = nc.vector.tensor_copy(shb[:], sh[:])

            # ---- W1 matmul: accumulate over jt K-tiles --------------------
            n_half = min(512, d1)
            n_banks = -(-d1 // n_half)
            pss = []
            for nb in range(n_banks):
                pst = pswpool.tile([_P, n_half], fp32, tag=f"ps{nb}", name=f"ps{nb}")
                pss.append(pst)
            w1mms = []
            for jg in range(jt // wgrp):
                w1t = w1ts[jg]
                for jl in range(wgrp):
                    j = jg * wgrp + jl
                    lhsT = shb[:, j * B : (j + 1) * B]
                    for nb in range(n_banks):
                        w1mms.append(
                            nc.tensor.matmul(
                                pss[nb][:B, :],
                                lhsT,
                                w1t[:, jl * d1 + nb * n_half : jl * d1 + (nb + 1) * n_half],
                                start=(j == 0),
                                stop=(j == jt - 1),
                            )
                        )

            h1 = perpool.tile([B, d1], fp32, tag="h1")
            h1copies = []
            for nb in range(n_banks):
                h1copies.append(
                    nc.vector.tensor_copy(
                        h1[:, nb * n_half : (nb + 1) * n_half], pss[nb][:B, :]
                    )
                )
            DMAQ_out = nc.sync.dma_start(h1_out.ap(), h1[:])

            # ---------------- static order pins ----------------
            def chain(seq):
                for a, b in zip(seq, seq[1:]):
                    pin(b, a)

            # DMA: table 0 first (route/sel slot in after its A half so the
            # first multiplies start as early as possible), gather-0 indices
            # right after, then table 1, remaining indices, tables, W1, out
            ft0, A0, B0 = tabs[0]["dmas"]
            DMAQ += [ft0, A0, route_d, sel_d, B0, idx_dmas[0]]
            DMAQ += tabs[1]["dmas"] + idx_dmas[1:]
            DMAQ += tabs[2]["dmas"] + tabs[3]["dmas"] + w1dmas + [DMAQ_out]
            chain(DMAQ)
            # Pool: strict alternation, next gather ahead of extraction
            POOLQ += [g1i[0], g1i[1], g2i[0], g1i[2], g2i[1], g1i[3],
                      g2i[2], g2i[3]]
            chain(POOLQ)
            # DVE: table muls for p+2 between scan(p) and scan(p+1);
            # reduces as soon as their extraction lands
            DVEQ += tabs[0]["muls"] + tabs[1]["muls"]
            DVEQ += [scans[0]] + tabs[2]["muls"] + [scans[1]] + tabs[3]["muls"]
            DVEQ += [reds[0][0], reds[0][1], scans[2], reds[1][0], reds[1][1]]
            DVEQ += [scans[3], reds[2][0], reds[2][1], reds[3][0], reds[3][1]]
            DVEQ += [shb_c] + h1copies
            chain(DVEQ)
            # PE: route matmuls in pass order, then sel matmuls, then W1
            PEQ += tabs[0]["prs"] + tabs[1]["prs"] + tabs[2]["prs"]
            PEQ += tabs[3]["prs"]
            for p in range(_NTAB):
                PEQ += reds[p][2]
            PEQ += w1mms
            chain(PEQ)

    nc.compile()
    return nc


# ---------------------------------------------------------------- host side
def _wrap16(streams):
    """[8, J] per-group streams -> [128, J//16] wrapped-16 layout."""
    ngrp, J = streams.shape
    assert ngrp == 8 and J % 16 == 0
    out = np.zeros((_P, J // 16), streams.dtype)
    for g in range(8):
        out[g * 16 : (g + 1) * 16, :] = streams[g].reshape(J // 16, 16).T
    return out


def _core_slices(snp_ids, node_seg):
    ids = np.asarray(snp_ids).astype(np.int64)
    seg = np.asarray(node_seg).astype(np.int64)
    gpc = N_GENES // N_CORES
    gene_starts = np.searchsorted(seg, np.arange(0, N_GENES + 1, gpc))
    return ids, seg, gpc, gene_starts


def _bucket_counts(ids_c, gene_c, uniq, Kc, gpc):
    """Per-(bucket, gene) even-padded counts. bucket = T*8 + g."""
    cpos = np.searchsorted(uniq, ids_c)
    cchunk = cpos // Kc
    bucketid = (cchunk // 16) * 8 + (cchunk % 8)
    key = bucketid * gpc + gene_c
    cnt = np.bincount(key, minlength=32 * gpc).reshape(32, gpc)
    pad_cnt = cnt + (cnt & 1)
    return cpos, cchunk, bucketid, key, cnt, pad_cnt


def pick_cfg(snp_ids, node_seg):
    """Host pass over the indices: global compact chunk size Kc and padded
    stream length J."""
    ids, seg, gpc, gene_starts = _core_slices(snp_ids, node_seg)
    Kc = 0
    uniqs = []
    for c in range(N_CORES):
        lo, hi = gene_starts[c], gene_starts[c + 1]
        uniq = np.unique(ids[lo:hi])
        uniqs.append(uniq)
        Kc = max(Kc, -(-len(uniq) // (_NCHUNK * 16)) * 16)
    J = 0
    for c in range(N_CORES):
        lo, hi = gene_starts[c], gene_starts[c + 1]
        gene_c = seg[lo:hi] - c * gpc
        _, _, _, _, _, pad_cnt = _bucket_counts(
            ids[lo:hi], gene_c, uniqs[c], Kc, gpc
        )
        J = max(J, 2 + int(pad_cnt.sum(axis=1).max()))
    J = -(-J // 16) * 16
    return Kc, J, uniqs


def prep_inputs(cfg, snp, snp_ids, node_seg, filters, W1, uniqs):
    """Index/metadata preprocessing + zero-padding + pure layout permutation;
    all value computation happens on device."""
    import ml_dtypes

    Kc, J, gpc, gpad = cfg["Kc"], cfg["J"], cfg["gpc"], cfg["gpad"]
    nspad, d1 = cfg["nspad"], cfg["d1"]
    n_cores = cfg["n_cores"]
    ZIDX = 2 * Kc  # zero column (even; pads point here)

    ids, seg, _, gene_starts = _core_slices(snp_ids, node_seg)
    snp = np.asarray(snp, np.float32)
    filters = np.asarray(filters, np.float32)
    W1f = np.asarray(W1, np.float32)

    # mean+replicate routing: prX[m, j] = (1/8) sum_r ft[s(m)X, r, j]
    # ft row q = s*8+r; routeA: s(q) == g(m) = m//16, routeB: s(q) == 8+g(m)
    route = np.zeros((_P, 2 * _P), ml_dtypes.bfloat16)
    for m in range(_P):
        g = m // 16
        route[g * 8 : g * 8 + 8, m] = 1.0 / N_FILT
        route[(8 + g) * 8 : (8 + g) * 8 + 8, _P + m] = 1.0 / N_FILT

    sel8 = np.zeros((_P, 8), ml_dtypes.bfloat16)
    for p in range(_P):
        sel8[p, p % 8] = 1.0

    per_core = []
    for c in range(n_cores):
        lo, hi = gene_starts[c], gene_starts[c + 1]
        ids_c = ids[lo:hi]
        gene_c = seg[lo:hi] - c * gpc
        uniq = uniqs[c]
        nu = len(uniq)
        assert nu <= _NCHUNK * Kc

        # compact value tables (pure permutation of inputs)
        snp_c = np.zeros((B, _NCHUNK * Kc), np.float32)
        snp_c[:, :nu] = snp[:, uniq]
        filt_c = np.zeros((N_FILT, _NCHUNK * Kc), np.float32)
        filt_c[:, :nu] = filters[:, uniq]

        # zero-split table layout, pre-laid in DRAM (pure permutation + zero
        # padding): row p = 16g+8h+b, pass-T block cols [h*Kc : (h+1)*Kc]
        # hold snp_c[b, (16T+g+8h)*Kc + j]; the other half and the trailing
        # zero column pair stay 0
        TW = 2 * Kc + 2
        snp_perm = np.zeros((_P, _NTAB * TW), np.float32)
        sp4 = snp_perm.reshape(8, 2, 8, _NTAB * TW)  # [g, h, b, cols]
        filt_perm = np.empty((_P, _NTAB * Kc), np.float32)
        for T in range(_NTAB):
            vi = snp_c[:, 16 * T * Kc : (16 * T + 16) * Kc].reshape(B, 2, 8, Kc)
            perm = vi.transpose(2, 1, 0, 3)  # [g, h, b, j]
            for h in range(2):
                sp4[:, h, :, T * TW + h * Kc : T * TW + (h + 1) * Kc] = perm[:, h]
            fi = filt_c[:, 16 * T * Kc : (16 * T + 16) * Kc].reshape(
                N_FILT, 16, Kc
            )
            filt_perm[:, T * Kc : (T + 1) * Kc] = fi.transpose(1, 0, 2).reshape(
                _P, Kc
            )
        filt_perm_bf = filt_perm.astype(ml_dtypes.bfloat16)

        cpos, cchunk, bucketid, key, cnt, pad_cnt = _bucket_counts(
            ids_c, gene_c, uniq, Kc, gpc
        )
        clidx = cpos % Kc
        # gene-ordered per-bucket streams with even per-gene padding
        order = np.argsort(bucketid, kind="stable")  # gene order preserved
        skey = key[order]
        stbl = (clidx[order] + np.where((cchunk[order] % 16) >= 8, Kc, 0)).astype(
            np.int64
        )
        flat_cnt = cnt.reshape(-1)
        flat_pad = pad_cnt.reshape(-1)
        starts = np.zeros(32 * gpc, np.int64)  # node start per key
        np.cumsum(flat_cnt[:-1], out=starts[1:])
        offs = np.zeros(32 * gpc, np.int64)  # padded stream offset per key
        pc = flat_pad.reshape(32, gpc)
        row_off = np.cumsum(pc, axis=1)
        offs = (
            2 + np.concatenate([np.zeros((32, 1), np.int64), row_off[:, :-1]], axis=1)
        ).reshape(-1)
        rank = np.arange(len(skey), dtype=np.int64) - starts[skey]
        pos = offs[skey] + rank
        streams = np.full((32, J), ZIDX, np.int16)
        streams[bucketid[order], pos] = stbl.astype(np.int16)
        tot = 2 + pc.sum(axis=1)
        assert int(tot.max()) <= J, f"bucket {int(tot.max())} exceeds J={J}"

        # boundaries (pair units): [0, end(g0), ..., end(g_{gpc-1})], pad
        ends = ((offs.reshape(32, gpc) + pc) // 2 - 1).astype(np.int16)
        ebnd = np.zeros((32, nspad), np.int16)
        ebnd[:, 1 : gpc + 1] = ends
        ebnd[:, gpc + 1 :] = ends[:, -1:]

        gidx_all = np.concatenate(
            [_wrap16(streams[T * 8 : (T + 1) * 8]) for T in range(_NTAB)], axis=1
        )
        eidx_all = np.concatenate(
            [_wrap16(ebnd[T * 8 : (T + 1) * 8]) for T in range(_NTAB)], axis=1
        )

        w1c = np.zeros((gpad, d1), np.float32)
        w1c[:gpc] = W1f[c * gpc : (c + 1) * gpc]
        jt_ = gpad // _P
        w1perm = np.ascontiguousarray(
            w1c.reshape(jt_, _P, d1).transpose(1, 0, 2).reshape(_P, jt_ * d1)
        ).astype(ml_dtypes.bfloat16)

        per_core.append(
            dict(
                snp_perm=snp_perm, filt_perm=filt_perm_bf, sel=sel8, w1c=w1perm,
                mroute=route, gidx=gidx_all, eidx=eidx_all,
            )
        )
    return per_core


def host_tail(h1_sum, b1, g1, be1, W2, b2, g2, be2, W3, b3, g3, be3,
              Wh1, bh1, gh, beh, Wh2, bh2):
    def bn(x, g, be):
        return x * (g / np.sqrt(np.float32(1.0 + BN_EPS))) + be

    relu = lambda x: np.maximum(x, np.float32(0.0))
    h = relu(bn(h1_sum + b1, g1, be1))
    h = relu(bn(h @ W2 + b2, g2, be2))
    feat = relu(bn(h @ W3 + b3, g3, be3))
    m = relu(bn(feat[:, :15] @ Wh1 + bh1, gh, beh))
    return (m @ Wh2 + bh2).astype(np.float32)


_CACHE = {}


def kernel(snp, snp_ids, node_seg, filters, W1, b1, g1, be1, W2, b2, g2, be2,
           W3, b3, g3, be3, Wh1, bh1, gh, beh, Wh2, bh2):
    from concourse import bass_utils

    Kc, J, uniqs = pick_cfg(snp_ids, node_seg)
    cfg = make_cfg(Kc, J)

    key = ("v2", Kc, J)
    if key not in _CACHE:
        _CACHE[key] = build_program(cfg)
    nc = _CACHE[key]

    in_maps = prep_inputs(cfg, snp, snp_ids, node_seg, filters, W1, uniqs)
    res = bass_utils.run_bass_kernel_spmd(
        nc, in_maps, core_ids=list(range(cfg["n_cores"]))
    )
    h1_sum = np.zeros((B, cfg["d1"]), np.float32)
    for c in range(cfg["n_cores"]):
        h1_sum += res.results[c]["h1p"]

    f32 = lambda x: np.asarray(x, np.float32)
    return host_tail(h1_sum, f32(b1), f32(g1), f32(be1), f32(W2), f32(b2),
                     f32(g2), f32(be2), f32(W3), f32(b3), f32(g3), f32(be3),
                     f32(Wh1), f32(bh1), f32(gh), f32(beh), f32(Wh2), f32(bh2))
